# revision 15
# baseline (speedup 1.0000x reference)
"""NVFP4-fake-quant MLP (x@w1.T -> gelu -> @w2.T) on 8 trn2 NeuronCores.

Sharding (megatron tensor-parallel on the hidden dim):
  core c holds w1 rows [c*2048:(c+1)*2048], w2 cols [c*2048:(c+1)*2048],
  and x rows [c*1024:(c+1)*1024] (for distributed x-quantization).

Exact quantization:
  per-16-block e4m3 scales via exponent-mask + magic-number RNE;
  fp4 e2m1 rounding via 3-region clamp + magic-round decomposition.
  e2m1_value * e4m3_blockscale has <= 6 mantissa bits -> stored EXACTLY in
  bf16, so the bf16 matmuls reproduce the f32 reference; per-tensor scales
  are folded into the PSUM->SBUF copies (gelu input scale / output scale).

Dataflow / overlap (v3):
  Prologue: w1-amax chunks + x-quant interleave from t=0; AR1 early; xqT
  transposes chase x-quant per column-chunk so the AllGather hides under
  w1-quant.  Quant pools are deep (bufs=4, no w1T reservation yet) so the
  ~19-op cross-engine quant chain pipelines; 4 of its ops (both clamp ops
  and both e2m1-reconstruction adds) run on GpSimd, splitting the
  elementwise load three ways (DVE/ACT/POOL).  w1T transposes + phase-1
  pools take over the region afterwards.
  Phase 1: w2 amax for t<16, AR2 at t=16, then only the FIRST half of w2
  (rows 0..2047) quantizes at 2 chunks/b-tile -- Vector stays under the
  PE period, no PSUM-drain stalls.
  Phase 2 runs in two column-half passes: pass A (out cols 0..2047) needs
  only w2T-half-A, and its idle Vector quantizes w2 rows 2048..4095;
  pass B computes cols 2048..4095 and fires one bf16 ReduceScatter per
  512-row chunk (16 total, spread evenly), each chunk's f32 cast-store
  issued by gpsimd DMA as soon as its collective lands.  Partials are
  bf16 throughout: half the HBM traffic and half the RS wire bytes.
"""
import os
import sys
import numpy as np

if "/opt/trn_rl_repo" not in sys.path:
    sys.path.insert(0, "/opt/trn_rl_repo")

f32 = np.float32

B, D_IN, HID, D_OUT = 8192, 4096, 16384, 4096
NCORES = 8
BSH = B // NCORES          # 1024 x-rows quantized per core
HSH = HID // NCORES        # 2048 hidden units per core
SB = 512                   # phase-2 transpose-load superblock rows
NSB = B // SB
NBT = B // 128             # 64 b-tiles
RSCH = 8                   # reduce-scatter chunks
RSROWS = B // RSCH         # 1024 rows per RS chunk
RSOUT = RSROWS // NCORES   # 64 rows per core per chunk
NK1 = D_IN // 128          # 32 k-tiles, first matmul
NK2 = HSH // 128           # 16 k-tiles, second matmul

# magic round-to-nearest-even constants (f32-exact)
C_HALF = float(f32(1.5 * 2 ** 22))       # grid 0.5
C_1 = float(f32(1.5 * 2 ** 23))          # grid 1
C_1B = float(f32(1.5 * 2 ** 23 + 2.0))   # C_1 + 2
C_2 = float(f32(1.5 * 2 ** 24))          # grid 2
C_2B = float(f32(1.5 * 2 ** 24 + 4.0))   # C_2 + 4
E4M3_MAGIC = float(f32(1.5 * 2 ** 20))   # * 2^e -> magic const for step 2^(e-3)
EXPMASK = 0x7F800000
SIGNMASK = 0x80000000
ONEBITS = 0x3F800000

_BUILT = {}
USE_GP = os.environ.get("KQ_USE_GPSIMD", "0") == "1"  # Pool ALU ~15x slower than DVE; keep off


def _emit_quant(nc, mybir, pf, pb, pn, biases, src, out, c1, effmul, W,
                signed=True):
    """Quantize src [128, W] f32 (SBUF) -> out [128, W] bf16 = sign*e2m1*bscale.

    c1: 1/(6*tensor_scale)  (float imm or [128,1] AP)
    effmul: tensor_scale    (float imm or [128,1] AP)
    biases: dict of [128,1] f32 bias tiles for the ACT magic rounds.

    signed=False is valid when every negative src value is guaranteed to
    quantize to 0 (gelu outputs: |neg| <= 0.17 < 0.375 <= 0.25*eff_min);
    the signed r then flows through the clamps/magic-rounds to exact 0s,
    saving the abs / sign-extract / sign-multiply ops.

"""
    OP = mybir.AluOpType
    AF = mybir.ActivationFunctionType
    U32 = mybir.dt.uint32
    FP32 = mybir.dt.float32
    BF16 = mybir.dt.bfloat16
    NB = W // 16
    ev = nc.gpsimd if USE_GP else nc.vector

    if signed:
        absv = pf.tile([128, W], FP32, tag="q_absv", name="q_absv")
        nc.scalar.activation(absv[:], src, AF.Abs)
    else:
        absv = None
    amax = pn.tile([128, NB], FP32, tag="q_amax", name="q_amax")
    nc.vector.tensor_reduce(
        amax[:],
        (absv[:] if signed else src).rearrange("p (nb b) -> p nb b", b=16),
        axis=mybir.AxisListType.X, op=OP.max,
        apply_absolute_value=(None if signed else True))
    vq = pn.tile([128, NB], FP32, tag="q_vq", name="q_vq")
    nc.vector.tensor_scalar(vq[:], amax[:], c1, None, OP.mult)
    scq = pn.tile([128, NB], FP32, tag="q_scq", name="q_scq")
    nc.vector.tensor_scalar(scq[:].bitcast(U32), vq[:].bitcast(U32),
                            EXPMASK, None, OP.bitwise_and)
    cb = pn.tile([128, NB], FP32, tag="q_cb", name="q_cb")
    nc.vector.tensor_scalar(cb[:], scq[:], E4M3_MAGIC, None, OP.mult)
    t4 = pn.tile([128, NB], FP32, tag="q_t4", name="q_t4")
    nc.vector.tensor_tensor(t4[:], vq[:], cb[:], OP.add)
    bs = pn.tile([128, NB], FP32, tag="q_bs", name="q_bs")
    nc.vector.tensor_tensor(bs[:], t4[:], cb[:], OP.subtract)
    bs16 = pn.tile([128, NB], BF16, tag="q_bs16", name="q_bs16")
    nc.vector.tensor_scalar(bs16[:], bs[:], 2.0 ** -6, None, OP.max)
    eff = pn.tile([128, NB], FP32, tag="q_eff", name="q_eff")
    nc.vector.tensor_scalar(eff[:], bs[:], 2.0 ** -6, effmul, OP.max, OP.mult)
    rec = pn.tile([128, NB], FP32, tag="q_rec", name="q_rec")
    nc.vector.reciprocal(rec[:], eff[:])
    r = pf.tile([128, W], FP32, tag="q_r", name="q_r")
    nc.vector.tensor_tensor(
        r[:].rearrange("p (nb b) -> p nb b", b=16),
        (absv[:] if signed else src).rearrange("p (nb b) -> p nb b", b=16),
        rec[:, :, None].to_broadcast([128, NB, 16]), OP.mult)
    m1 = pf.tile([128, W], FP32, tag="q_absv", name="q_m1")   # reuse absv slots
    nc.vector.tensor_scalar(m1[:], r[:], 2.0, None, OP.min)
    m3 = pf.tile([128, W], FP32, tag="q_m23", name="q_m3")
    ev.tensor_scalar(m3[:], r[:], 4.0, 6.0, OP.max, OP.min)
    nc.scalar.activation(m3[:], m3[:], AF.Identity, bias=biases["c2"][:])
    s3 = pb.tile([128, W], BF16, tag="q_s3", name="q_s3")
    nc.scalar.activation(s3[:], m3[:], AF.Identity, bias=biases["nc2b"][:])
    m2 = pf.tile([128, W], FP32, tag="q_m23", name="q_m2")
    ev.tensor_scalar(m2[:], r[:], 2.0, 4.0, OP.max, OP.min)
    nc.scalar.activation(m2[:], m2[:], AF.Identity, bias=biases["c1"][:])
    s2 = pb.tile([128, W], BF16, tag="q_s2", name="q_s2")
    nc.scalar.activation(s2[:], m2[:], AF.Identity, bias=biases["nc1b"][:])
    nc.scalar.activation(m1[:], m1[:], AF.Identity, bias=biases["ch"][:])
    s1 = pb.tile([128, W], BF16, tag="q_s1", name="q_s1")
    nc.scalar.activation(s1[:], m1[:], AF.Identity, bias=biases["nch"][:])
    q12 = pb.tile([128, W], BF16, tag="q_s1", name="q_q12")
    ev.tensor_tensor(q12[:], s1[:], s2[:], OP.add)
    qq = pb.tile([128, W], BF16, tag="q_s2", name="q_qq")
    ev.tensor_tensor(qq[:], q12[:], s3[:], OP.add)
    if signed:
        qs = pb.tile([128, W], BF16, tag="q_s1", name="q_qs")
        nc.vector.tensor_tensor(
            qs[:].rearrange("p (nb b) -> p nb b", b=16),
            qq[:].rearrange("p (nb b) -> p nb b", b=16),
            bs16[:, :, None].to_broadcast([128, NB, 16]), OP.mult)
        sgn = pf.tile([128, W], FP32, tag="q_r", name="q_sgn")  # reuse r slots
        nc.vector.tensor_scalar(sgn[:].bitcast(U32), src.bitcast(U32),
                                SIGNMASK, ONEBITS,
                                OP.bitwise_and, OP.bitwise_or)
        nc.vector.tensor_tensor(out, qs[:], sgn[:], OP.mult)
    else:
        nc.vector.tensor_tensor(
            out.rearrange("p (nb b) -> p nb b", b=16),
            qq[:].rearrange("p (nb b) -> p nb b", b=16),
            bs16[:, :, None].to_broadcast([128, NB, 16]), OP.mult)


def _build(isc, hsc):
    from contextlib import ExitStack
    import concourse.bass as bass
    import concourse.tile as tile
    from concourse import bacc, mybir

    OP = mybir.AluOpType
    AF = mybir.ActivationFunctionType
    FP32 = mybir.dt.float32
    BF16 = mybir.dt.bfloat16

    c1x = float(f32(1.0) / (f32(6.0) * f32(isc)))
    c1h = float(f32(1.0) / (f32(6.0) * f32(hsc)))
    inv2688 = float(f32(1.0) / f32(2688.0))
    RG = [list(range(NCORES))]

    nc = bacc.Bacc("TRN2", target_bir_lowering=False, debug=False,
                   num_devices=NCORES)
    x_sh = nc.dram_tensor("x_sh", [BSH, D_IN], FP32, kind="ExternalInput").ap()
    w1_sh = nc.dram_tensor("w1_sh", [HSH, D_IN], FP32, kind="ExternalInput").ap()
    w2_sh = nc.dram_tensor("w2_sh", [D_OUT, HSH], FP32, kind="ExternalInput").ap()
    out_sh = nc.dram_tensor("out_sh", [BSH, D_OUT], FP32, kind="ExternalOutput").ap()

    with tile.TileContext(nc) as tc, ExitStack() as top:
        dram = top.enter_context(tc.tile_pool(name="dram", bufs=1, space="DRAM"))
        amax_stage = dram.tile([128, 2], FP32, tag="amax_stage", name="amax_stage")
        s1loc = dram.tile([1, 1], FP32, tag="s1loc", name="s1loc")
        s2loc = dram.tile([1, 1], FP32, tag="s2loc", name="s2loc")
        s1sh = dram.tile([1, 1], FP32, tag="s1sh", name="s1sh", addr_space="Shared")
        s2sh = dram.tile([1, 1], FP32, tag="s2sh", name="s2sh", addr_space="Shared")
        xq_loc = dram.tile([BSH, D_IN], BF16, tag="xq_loc", name="xq_loc")
        xqT_loc = dram.tile([D_IN, BSH], BF16, tag="xqT_loc", name="xqT_loc")
        xqT_full = dram.tile([NCORES * D_IN, BSH], BF16, tag="xqT_full",
                             name="xqT_full", addr_space="Shared")
        w1q = dram.tile([HSH, D_IN], BF16, tag="w1q", name="w1q")
        w2q = dram.tile([D_OUT, HSH], BF16, tag="w2q", name="w2q")
        hq = dram.tile([B, HSH], BF16, tag="hq", name="hq")
        parts = [dram.tile([RSROWS, D_OUT], BF16, name=f"part{c}", tag=f"part{c}")
                 for c in range(RSCH)]
        rsouts = [dram.tile([RSOUT, D_OUT], BF16, name=f"rsout{c}",
                            tag=f"rsout{c}")
                  for c in range(RSCH)]

        singles = top.enter_context(tc.tile_pool(name="singles", bufs=1))
        biases = {}
        for nm, val in [("ch", C_HALF), ("nch", -C_HALF),
                        ("c1", C_1), ("nc1b", -C_1B),
                        ("c2", C_2), ("nc2b", -C_2B)]:
            bt = singles.tile([128, 1], FP32, tag=f"bias_{nm}", name=f"bias_{nm}")
            nc.vector.memset(bt[:], val)
            biases[nm] = bt

        # ================= Phase 0 =================
        # deep-buffered quant pipelines get the whole SBUF; w1T takes the
        # region over afterwards for its transposes + phase 1.
        with tc.tile_pool(name="p0src", bufs=4) as p0src, \
             tc.tile_pool(name="p0f", bufs=4) as p0f, \
             tc.tile_pool(name="p0b", bufs=4) as p0b, \
             tc.tile_pool(name="p0n", bufs=4) as p0n:
            acc1 = singles.tile([128, 1], FP32, tag="acc1", name="acc1")

            def w1_amax_chunk(i):
                # one [128, 1024] quarter-row-block of w1 -> running |.|max
                wt = p0src.tile([128, 1024], FP32, tag="wamax", name="wamax",
                                bufs=3)
                nc.scalar.dma_start(
                    wt[:], w1_sh[(i // 4) * 128:(i // 4 + 1) * 128,
                                 (i % 4) * 1024:(i % 4 + 1) * 1024])
                am = p0n.tile([128, 1], FP32, tag="am_w", name="am_w")
                nc.vector.tensor_reduce(am[:], wt[:],
                                        axis=mybir.AxisListType.X,
                                        op=OP.max, apply_absolute_value=True)
                if i == 0:
                    nc.vector.tensor_copy(acc1[:], am[:])
                else:
                    nc.vector.tensor_tensor(acc1[:], acc1[:], am[:], OP.max)

            def x_quant_tile(i, c):
                sl = slice(c * 1024, (c + 1) * 1024)
                xt = p0src.tile([128, 1024], FP32, tag="xt", name="xt")
                nc.scalar.dma_start(xt[:], x_sh[i * 128:(i + 1) * 128, sl])
                xo = p0src.tile([128, 1024], BF16, tag="xo", name="xo")
                _emit_quant(nc, mybir, p0f, p0b, p0n, biases,
                            xt[:], xo[:], c1x, float(isc), 1024)
                nc.sync.dma_start(xq_loc[i * 128:(i + 1) * 128, sl], xo[:])

            # ---- interleave the w1-amax stream with x-quant: one x tile
            # per 4 amax chunks keeps Vector saturated while the 32MB amax
            # stream flows (covers x col-chunks 0 and 1) ----
            for i in range(16):
                for u in range(4):
                    w1_amax_chunk(4 * i + u)
                x_quant_tile(i // 2, i % 2)
            # finish w1-amax reduction -> AR1 (gpsimd ring slot #1)
            nc.sync.dma_start(amax_stage[:, 0:1], acc1[:])
            rowv1 = singles.tile([1, 128], FP32, tag="rowv1", name="rowv1")
            nc.sync.dma_start(
                rowv1[:], amax_stage[:, 0:1].rearrange("p c -> (p c)").unsqueeze(0))
            red1 = singles.tile([1, 1], FP32, tag="red1", name="red1")
            nc.vector.tensor_reduce(red1[:], rowv1[:],
                                    axis=mybir.AxisListType.X, op=OP.max)
            nc.sync.dma_start(s1loc[:], red1[:])
            nc.gpsimd.collective_compute(
                "AllReduce", OP.max, replica_groups=RG,
                ins=[s1loc[:].opt()], outs=[s1sh[:].opt()])
            sam1 = singles.tile([128, 1], FP32, tag="sam1", name="sam1")
            ap1 = s1sh[:]
            nc.gpsimd.dma_start(sam1[:], bass.AP(
                tensor=ap1.tensor, offset=ap1.offset,
                ap=[[0, 128]] + list(ap1.ap)[1:]))

            # ---- rest of x-quant, col-major; xqT transposes chase each
            # finished column-chunk ----
            def xqT_transpose(k):
                xtt = p0src.tile([128, BSH], BF16, tag="xtt", name="xtt",
                                 bufs=2)
                nc.sync.dma_start(xtt[:], xq_loc[:, k * 128:(k + 1) * 128],
                                  transpose=True)
                nc.sync.dma_start(xqT_loc[k * 128:(k + 1) * 128, :], xtt[:])

            for k in range(16):          # col-chunks 0+1 finished above
                xqT_transpose(k)
            for c in range(2, 4):
                for i in range(8):
                    x_quant_tile(i, c)
                for k in range(8 * c, 8 * c + 8):
                    xqT_transpose(k)
            nc.gpsimd.collective_compute(
                "AllGather", OP.bypass, replica_groups=RG,
                ins=[xqT_loc[:].opt()], outs=[xqT_full[:].opt()])

            # ---- w1 scale scalars (AR1 has long landed) ----
            tsw1 = singles.tile([128, 1], FP32, tag="tsw1", name="tsw1")
            nc.vector.tensor_scalar(tsw1[:], sam1[:], inv2688, None, OP.mult)
            dw1 = singles.tile([128, 1], FP32, tag="dw1", name="dw1")
            nc.vector.tensor_scalar(dw1[:], tsw1[:], 6.0, None, OP.mult)
            rdw1 = singles.tile([128, 1], FP32, tag="rdw1", name="rdw1")
            nc.vector.reciprocal(rdw1[:], dw1[:])
            s_h = singles.tile([128, 1], FP32, tag="s_h", name="s_h")
            nc.vector.tensor_scalar(s_h[:], tsw1[:], float(isc), None, OP.mult)

            # ---- quantize w1 (col-major; w1q lands in DRAM) ----
            for c in range(4):
                sl = slice(c * 1024, (c + 1) * 1024)
                for j in range(HSH // 128):
                    wt = p0src.tile([128, 1024], FP32, tag="xt", name="wt")
                    nc.scalar.dma_start(wt[:], w1_sh[j * 128:(j + 1) * 128, sl])
                    wo = p0src.tile([128, 1024], BF16, tag="xo", name="wo")
                    _emit_quant(nc, mybir, p0f, p0b, p0n, biases,
                                wt[:], wo[:], rdw1[:], tsw1[:], 1024)
                    nc.sync.dma_start(w1q[j * 128:(j + 1) * 128, sl], wo[:])

        # w1T region takes over the freed quant scratch; its transposes
        # run as soon as the w1q stores land.
        w1T_cm = tc.tile_pool(name="w1T", bufs=1)
        w1T_pool = w1T_cm.__enter__()
        w1T = w1T_pool.tile([128, NK1, HSH], BF16, tag="w1T", name="w1T")
        for k in range(NK1):
            nc.sync.dma_start(w1T[:, k, :], w1q[:, k * 128:(k + 1) * 128],
                              transpose=True)

        # ================= Phase 1 =================
        with tc.tile_pool(name="xb", bufs=2) as xb_pool, \
             tc.tile_pool(name="q1f", bufs=2) as q1f, \
             tc.tile_pool(name="q1b", bufs=2) as q1b, \
             tc.tile_pool(name="q1n", bufs=2) as q1n, \
             tc.tile_pool(name="w2s", bufs=2) as w2s, \
             tc.tile_pool(name="ps1", bufs=8, space="PSUM") as ps1:
            acc2 = singles.tile([128, 1], FP32, tag="acc2", name="acc2")
            tsw2 = singles.tile([128, 1], FP32, tag="tsw2", name="tsw2")
            rdw2 = singles.tile([128, 1], FP32, tag="rdw2", name="rdw2")
            s_o = singles.tile([128, 1], FP32, tag="s_o", name="s_o")

            # ---- main phase-1 loop; w2 amax / AR2 / half-quant mixed in ----
            for t in range(NBT):
                g0 = t * 128
                ci, off = divmod(g0, BSH)
                xb = xb_pool.tile([128, NK1, 128], BF16, tag="xb", name="xb")
                nc.sync.dma_start(
                    xb[:],
                    xqT_full[ci * D_IN:(ci + 1) * D_IN, off:off + 128]
                    .rearrange("(k p) c -> p k c", p=128))
                pss = [ps1.tile([128, 512], FP32, name="ps", tag="ps")
                       for _ in range(4)]
                for k in range(NK1):
                    for n in range(4):
                        nc.tensor.matmul(
                            pss[n][:], lhsT=xb[:, k, :],
                            rhs=w1T[:, k, n * 512:(n + 1) * 512],
                            start=(k == 0), stop=(k == NK1 - 1))
                for half in range(2):
                    g = q1f.tile([128, 1024], FP32, tag="q_g", name="q_g")
                    ho = q1b.tile([128, 1024], BF16, tag="q_ho", name="q_ho")
                    for n2 in range(2):
                        nc.scalar.activation(
                            g[:, n2 * 512:(n2 + 1) * 512],
                            pss[half * 2 + n2][:], AF.Gelu, scale=s_h[:])
                    _emit_quant(nc, mybir, q1f, q1b, q1n, biases,
                                g[:], ho[:], c1h, float(hsc), 1024,
                                signed=False)
                    nc.sync.dma_start(
                        hq[g0:g0 + 128, half * 1024:(half + 1) * 1024], ho[:])
                if t < 16:
                    # w2 amax stream: 4 x [128, 512] chunks per b-tile
                    for u in range(4):
                        i2 = 4 * t + u
                        wt3 = w2s.tile([128, 512], FP32, tag="wt2",
                                       name="wt3")
                        nc.scalar.dma_start(
                            wt3[:],
                            w2_sh[(i2 // 4) * 128:(i2 // 4 + 1) * 128,
                                  (i2 % 4) * 512:(i2 % 4 + 1) * 512])
                        am2 = q1n.tile([128, 1], FP32, tag="am_w2", name="am_w2")
                        nc.vector.tensor_reduce(am2[:], wt3[:],
                                                axis=mybir.AxisListType.X,
                                                op=OP.max,
                                                apply_absolute_value=True)
                        if i2 == 0:
                            nc.vector.tensor_copy(acc2[:], am2[:])
                        else:
                            nc.vector.tensor_tensor(acc2[:], acc2[:], am2[:],
                                                    OP.max)
                elif t == 16:
                    # all 64 w2-amax chunks are in; AR2 + scale scalars
                    nc.sync.dma_start(amax_stage[:, 1:2], acc2[:])
                    rowv2 = singles.tile([1, 128], FP32, tag="rowv2",
                                         name="rowv2")
                    nc.sync.dma_start(
                        rowv2[:],
                        amax_stage[:, 1:2].rearrange("p c -> (p c)").unsqueeze(0))
                    red2 = singles.tile([1, 1], FP32, tag="red2", name="red2")
                    nc.vector.tensor_reduce(red2[:], rowv2[:],
                                            axis=mybir.AxisListType.X, op=OP.max)
                    nc.sync.dma_start(s2loc[:], red2[:])
                    nc.gpsimd.collective_compute(
                        "AllReduce", OP.max, replica_groups=RG,
                        ins=[s2loc[:].opt()], outs=[s2sh[:].opt()])
                    sam2 = singles.tile([128, 1], FP32, tag="sam2", name="sam2")
                    ap2 = s2sh[:]
                    nc.gpsimd.dma_start(sam2[:], bass.AP(
                        tensor=ap2.tensor, offset=ap2.offset,
                        ap=[[0, 128]] + list(ap2.ap)[1:]))
                    nc.vector.tensor_scalar(tsw2[:], sam2[:], inv2688, None,
                                            OP.mult)
                    dw2 = singles.tile([128, 1], FP32, tag="dw2", name="dw2")
                    nc.vector.tensor_scalar(dw2[:], tsw2[:], 6.0, None, OP.mult)
                    nc.vector.reciprocal(rdw2[:], dw2[:])
                    nc.vector.tensor_scalar(s_o[:], tsw2[:], float(hsc), None,
                                            OP.mult)
                elif 17 <= t < 49:
                    # quantize only w2 rows 0..2047 here (2 chunks/b-tile);
                    # rows 2048..4095 quantize during phase-2 pass A.
                    for u in range(2):
                        i2 = 2 * (t - 17) + u
                        wi, cc = divmod(i2, 4)
                        sl = slice(cc * 512, (cc + 1) * 512)
                        wt2 = w2s.tile([128, 512], FP32, tag="wt2",
                                       name="wt2")
                        nc.scalar.dma_start(
                            wt2[:], w2_sh[wi * 128:(wi + 1) * 128, sl])
                        wo2 = w2s.tile([128, 512], BF16, tag="wo2",
                                       name="wo2")
                        _emit_quant(nc, mybir, q1f, q1b, q1n, biases,
                                    wt2[:], wo2[:], rdw2[:], tsw2[:],
                                    512)
                        nc.sync.dma_start(
                            w2q[wi * 128:(wi + 1) * 128, sl], wo2[:])

        # ================= Phase 2 =================
        w1T_cm.__exit__(None, None, None)
        with tc.tile_pool(name="w2Ta", bufs=1) as w2Ta_pool, \
             tc.tile_pool(name="w2Tb", bufs=1) as w2Tb_pool, \
             tc.tile_pool(name="hT", bufs=2) as hT_pool, \
             tc.tile_pool(name="osb", bufs=2) as osb, \
             tc.tile_pool(name="q2f", bufs=3) as q2f, \
             tc.tile_pool(name="q2b", bufs=2) as q2b, \
             tc.tile_pool(name="q2n", bufs=3) as q2n, \
             tc.tile_pool(name="w2s2", bufs=2) as w2s2, \
             tc.tile_pool(name="ps2", bufs=8, space="PSUM") as ps2:
            w2Ta = w2Ta_pool.tile([128, NK2, 2048], BF16, tag="w2Ta",
                                  name="w2Ta")
            w2Tb = w2Tb_pool.tile([128, NK2, 2048], BF16, tag="w2Tb",
                                  name="w2Tb")
            for k in range(NK2):
                nc.sync.dma_start(w2Ta[:, k, :],
                                  w2q[0:2048, k * 128:(k + 1) * 128],
                                  transpose=True)

            def p2_pass(colh, w2T):
                for sb in range(NSB):
                    r0 = sb * SB
                    hT = hT_pool.tile([128, NK2, SB], BF16, tag="hT",
                                      name="hT")
                    for k in range(NK2):
                        nc.sync.dma_start(
                            hT[:, k, :],
                            hq[r0:r0 + SB, k * 128:(k + 1) * 128],
                            transpose=True)
                    for b in range(SB // 128):
                        row = r0 + b * 128
                        c = row // RSROWS
                        crow = row % RSROWS
                        pss = [ps2.tile([128, 512], FP32, name="ps2",
                                        tag="ps2")
                               for _ in range(4)]
                        for k in range(NK2):
                            for n in range(4):
                                nc.tensor.matmul(
                                    pss[n][:],
                                    lhsT=hT[:, k, b * 128:(b + 1) * 128],
                                    rhs=w2T[:, k, n * 512:(n + 1) * 512],
                                    start=(k == 0), stop=(k == NK2 - 1))
                        ot = osb.tile([128, 2048], BF16, tag="ot", name="ot")
                        for n in range(4):
                            nc.scalar.activation(ot[:, n * 512:(n + 1) * 512],
                                                 pss[n][:], AF.Copy,
                                                 scale=s_o[:])
                        nc.sync.dma_start(
                            parts[c][crow:crow + 128,
                                     colh * 2048:(colh + 1) * 2048], ot[:])
                    if colh == 0:
                        # pass A: idle Vector quantizes w2 rows 2048..4095
                        for u in range(4):
                            i2 = 4 * sb + u
                            wi, cc = divmod(i2, 4)
                            wi += 16
                            sl = slice(cc * 512, (cc + 1) * 512)
                            wt2 = w2s2.tile([128, 512], FP32, tag="wt2b",
                                            name="wt2b")
                            nc.scalar.dma_start(
                                wt2[:], w2_sh[wi * 128:(wi + 1) * 128, sl])
                            wo2 = w2s2.tile([128, 512], BF16, tag="wo2b",
                                            name="wo2b")
                            _emit_quant(nc, mybir, q2f, q2b, q2n, biases,
                                        wt2[:], wo2[:], rdw2[:], tsw2[:],
                                        512)
                            nc.sync.dma_start(
                                w2q[wi * 128:(wi + 1) * 128, sl], wo2[:])
                    elif sb % 2 == 1:
                        # pass B: chunk rows complete -> ReduceScatter
                        cch = sb // 2
                        nc.gpsimd.collective_compute(
                            "ReduceScatter", OP.add, replica_groups=RG,
                            ins=[parts[cch][:].opt()],
                            outs=[rsouts[cch][:].opt()])

                if colh == 0:
                    for k in range(NK2):
                        nc.sync.dma_start(
                            w2Tb[:, k, :],
                            w2q[2048:4096, k * 128:(k + 1) * 128],
                            transpose=True)

            p2_pass(0, w2Ta)
            p2_pass(1, w2Tb)
            # per-chunk f32 casts of the landed rs outputs (SWDGE cast-DMA);
            # chunk c's store fires as soon as its collective completes.
            for c in range(RSCH):
                nc.gpsimd.dma_start(out_sh[c * RSOUT:(c + 1) * RSOUT, :],
                                    rsouts[c][:])
    nc.compile()
    return nc


def _get_built(isc, hsc):
    key = (float(isc), float(hsc), USE_GP)
    if key not in _BUILT:
        _BUILT[key] = _build(float(isc), float(hsc))
    return _BUILT[key]


def run(x, w1, w2, input_scale, hidden_scale, trace=False):
    from concourse import bass_utils
    isc = float(np.asarray(input_scale).reshape(-1)[0])
    hsc = float(np.asarray(hidden_scale).reshape(-1)[0])
    nc = _get_built(isc, hsc)
    x = np.ascontiguousarray(x, dtype=np.float32)
    w1 = np.ascontiguousarray(w1, dtype=np.float32)
    w2 = np.ascontiguousarray(w2, dtype=np.float32)
    in_maps = []
    for c in range(NCORES):
        in_maps.append({
            "x_sh": x[c * BSH:(c + 1) * BSH, :],
            "w1_sh": np.ascontiguousarray(w1[c * HSH:(c + 1) * HSH, :]),
            "w2_sh": np.ascontiguousarray(w2[:, c * HSH:(c + 1) * HSH]),
        })
    res = bass_utils.run_bass_kernel_spmd(
        nc, in_maps, core_ids=list(range(NCORES)), trace=trace)
    out = np.empty((B, D_OUT), dtype=np.float32)
    for r in range(NCORES):
        o = res.results[r]["out_sh"]
        for c in range(RSCH):
            out[c * RSROWS + r * RSOUT:c * RSROWS + (r + 1) * RSOUT, :] = \
                o[c * RSOUT:(c + 1) * RSOUT, :]
    return out, res


def kernel(x, w1, w2, input_scale, hidden_scale):
    out, _ = run(x, w1, w2, input_scale, hidden_scale, trace=False)
    return out


# revision 17
# speedup vs baseline: 1.0160x; 1.0160x over previous
"""NVFP4-fake-quant MLP (x@w1.T -> gelu -> @w2.T) on 8 trn2 NeuronCores.

Sharding (megatron tensor-parallel on the hidden dim):
  core c holds w1 rows [c*2048:(c+1)*2048], w2 cols [c*2048:(c+1)*2048],
  and x rows [c*1024:(c+1)*1024] (for distributed x-quantization).

Exact quantization:
  per-16-block e4m3 scales via exponent-mask + magic-number RNE;
  fp4 e2m1 rounding via 3-region clamp + magic-round decomposition.
  e2m1_value * e4m3_blockscale has <= 6 mantissa bits -> stored EXACTLY in
  bf16, so the bf16 matmuls reproduce the f32 reference; per-tensor scales
  are folded into the PSUM->SBUF copies (gelu input scale / output scale).

Dataflow / overlap (v3):
  Prologue: w1-amax chunks + x-quant interleave from t=0; AR1 early; xqT
  transposes chase x-quant per column-chunk so the AllGather hides under
  w1-quant.  Quant pools are deep (bufs=4, no w1T reservation yet) so the
  ~19-op cross-engine DVE/ACT quant chain pipelines; w1T transposes +
  phase-1 pools take over the region afterwards.
  Phase 1: w2 amax for t<16, AR2 at t=16, then only the FIRST half of w2
  (rows 0..2047) quantizes at 2 chunks/b-tile -- Vector stays under the
  PE period, no PSUM-drain stalls.
  Phase 2 runs in two column-half passes: pass A (out cols 0..2047) needs
  only w2T-half-A, and its idle Vector quantizes w2 rows 2048..4095;
  pass B computes cols 2048..4095 and fires one bf16 ReduceScatter per
  512-row chunk (16 total, spread evenly), each chunk's f32 cast-store
  issued by gpsimd DMA as soon as its collective lands.  Partials are
  bf16 throughout: half the HBM traffic and half the RS wire bytes.
"""
import os
import sys
import numpy as np

if "/opt/trn_rl_repo" not in sys.path:
    sys.path.insert(0, "/opt/trn_rl_repo")

f32 = np.float32

B, D_IN, HID, D_OUT = 8192, 4096, 16384, 4096
NCORES = 8
BSH = B // NCORES          # 1024 x-rows quantized per core
HSH = HID // NCORES        # 2048 hidden units per core
SB = 512                   # phase-2 transpose-load superblock rows
NSB = B // SB
NBT = B // 128             # 64 b-tiles
RSCH = 16                  # reduce-scatter chunks
RSROWS = B // RSCH         # 512 rows per RS chunk
RSOUT = RSROWS // NCORES   # 64 rows per core per chunk
NK1 = D_IN // 128          # 32 k-tiles, first matmul
NK2 = HSH // 128           # 16 k-tiles, second matmul

# magic round-to-nearest-even constants (f32-exact)
C_HALF = float(f32(1.5 * 2 ** 22))       # grid 0.5
C_1 = float(f32(1.5 * 2 ** 23))          # grid 1
C_1B = float(f32(1.5 * 2 ** 23 + 2.0))   # C_1 + 2
C_2 = float(f32(1.5 * 2 ** 24))          # grid 2
C_2B = float(f32(1.5 * 2 ** 24 + 4.0))   # C_2 + 4
E4M3_MAGIC = float(f32(1.5 * 2 ** 20))   # * 2^e -> magic const for step 2^(e-3)
EXPMASK = 0x7F800000
SIGNMASK = 0x80000000
ONEBITS = 0x3F800000

_BUILT = {}
USE_GP = os.environ.get("KQ_USE_GPSIMD", "0") == "1"  # Pool ALU ~15x slower than DVE; keep off


def _emit_quant(nc, mybir, pf, pb, pn, biases, src, out, c1, effmul, W,
                signed=True):
    """Quantize src [128, W] f32 (SBUF) -> out [128, W] bf16 = sign*e2m1*bscale.

    c1: 1/(6*tensor_scale)  (float imm or [128,1] AP)
    effmul: tensor_scale    (float imm or [128,1] AP)
    biases: dict of [128,1] f32 bias tiles for the ACT magic rounds.

    signed=False is valid when every negative src value is guaranteed to
    quantize to 0 (gelu outputs: |neg| <= 0.17 < 0.375 <= 0.25*eff_min);
    the signed r then flows through the clamps/magic-rounds to exact 0s,
    saving the abs / sign-extract / sign-multiply ops.

"""
    OP = mybir.AluOpType
    AF = mybir.ActivationFunctionType
    U32 = mybir.dt.uint32
    FP32 = mybir.dt.float32
    BF16 = mybir.dt.bfloat16
    NB = W // 16
    ev = nc.gpsimd if USE_GP else nc.vector

    if signed:
        absv = pf.tile([128, W], FP32, tag="q_absv", name="q_absv")
        nc.scalar.activation(absv[:], src, AF.Abs)
    else:
        absv = None
    amax = pn.tile([128, NB], FP32, tag="q_amax", name="q_amax")
    nc.vector.tensor_reduce(
        amax[:],
        (absv[:] if signed else src).rearrange("p (nb b) -> p nb b", b=16),
        axis=mybir.AxisListType.X, op=OP.max,
        apply_absolute_value=(None if signed else True))
    vq = pn.tile([128, NB], FP32, tag="q_vq", name="q_vq")
    nc.vector.tensor_scalar(vq[:], amax[:], c1, None, OP.mult)
    scq = pn.tile([128, NB], FP32, tag="q_scq", name="q_scq")
    nc.vector.tensor_scalar(scq[:].bitcast(U32), vq[:].bitcast(U32),
                            EXPMASK, None, OP.bitwise_and)
    cb = pn.tile([128, NB], FP32, tag="q_cb", name="q_cb")
    nc.vector.tensor_scalar(cb[:], scq[:], E4M3_MAGIC, None, OP.mult)
    t4 = pn.tile([128, NB], FP32, tag="q_t4", name="q_t4")
    nc.vector.tensor_tensor(t4[:], vq[:], cb[:], OP.add)
    bs = pn.tile([128, NB], FP32, tag="q_bs", name="q_bs")
    nc.vector.tensor_tensor(bs[:], t4[:], cb[:], OP.subtract)
    bs16 = pn.tile([128, NB], BF16, tag="q_bs16", name="q_bs16")
    nc.vector.tensor_scalar(bs16[:], bs[:], 2.0 ** -6, None, OP.max)
    eff = pn.tile([128, NB], FP32, tag="q_eff", name="q_eff")
    nc.vector.tensor_scalar(eff[:], bs[:], 2.0 ** -6, effmul, OP.max, OP.mult)
    rec = pn.tile([128, NB], FP32, tag="q_rec", name="q_rec")
    nc.vector.reciprocal(rec[:], eff[:])
    r = pf.tile([128, W], FP32, tag="q_r", name="q_r")
    nc.vector.tensor_tensor(
        r[:].rearrange("p (nb b) -> p nb b", b=16),
        (absv[:] if signed else src).rearrange("p (nb b) -> p nb b", b=16),
        rec[:, :, None].to_broadcast([128, NB, 16]), OP.mult)
    m1 = pf.tile([128, W], FP32, tag="q_absv", name="q_m1")   # reuse absv slots
    nc.vector.tensor_scalar(m1[:], r[:], 2.0, None, OP.min)
    m3 = pf.tile([128, W], FP32, tag="q_m23", name="q_m3")
    ev.tensor_scalar(m3[:], r[:], 4.0, 6.0, OP.max, OP.min)
    nc.scalar.activation(m3[:], m3[:], AF.Identity, bias=biases["c2"][:])
    s3 = pb.tile([128, W], BF16, tag="q_s3", name="q_s3")
    nc.scalar.activation(s3[:], m3[:], AF.Identity, bias=biases["nc2b"][:])
    m2 = pf.tile([128, W], FP32, tag="q_m23", name="q_m2")
    ev.tensor_scalar(m2[:], r[:], 2.0, 4.0, OP.max, OP.min)
    nc.scalar.activation(m2[:], m2[:], AF.Identity, bias=biases["c1"][:])
    s2 = pb.tile([128, W], BF16, tag="q_s2", name="q_s2")
    nc.scalar.activation(s2[:], m2[:], AF.Identity, bias=biases["nc1b"][:])
    nc.scalar.activation(m1[:], m1[:], AF.Identity, bias=biases["ch"][:])
    s1 = pb.tile([128, W], BF16, tag="q_s1", name="q_s1")
    nc.scalar.activation(s1[:], m1[:], AF.Identity, bias=biases["nch"][:])
    q12 = pb.tile([128, W], BF16, tag="q_s1", name="q_q12")
    ev.tensor_tensor(q12[:], s1[:], s2[:], OP.add)
    qq = pb.tile([128, W], BF16, tag="q_s2", name="q_qq")
    ev.tensor_tensor(qq[:], q12[:], s3[:], OP.add)
    if signed:
        qs = pb.tile([128, W], BF16, tag="q_s1", name="q_qs")
        nc.vector.tensor_tensor(
            qs[:].rearrange("p (nb b) -> p nb b", b=16),
            qq[:].rearrange("p (nb b) -> p nb b", b=16),
            bs16[:, :, None].to_broadcast([128, NB, 16]), OP.mult)
        sgn = pf.tile([128, W], FP32, tag="q_r", name="q_sgn")  # reuse r slots
        nc.vector.tensor_scalar(sgn[:].bitcast(U32), src.bitcast(U32),
                                SIGNMASK, ONEBITS,
                                OP.bitwise_and, OP.bitwise_or)
        nc.vector.tensor_tensor(out, qs[:], sgn[:], OP.mult)
    else:
        nc.vector.tensor_tensor(
            out.rearrange("p (nb b) -> p nb b", b=16),
            qq[:].rearrange("p (nb b) -> p nb b", b=16),
            bs16[:, :, None].to_broadcast([128, NB, 16]), OP.mult)


def _build(isc, hsc):
    from contextlib import ExitStack
    import concourse.bass as bass
    import concourse.tile as tile
    from concourse import bacc, mybir

    OP = mybir.AluOpType
    AF = mybir.ActivationFunctionType
    FP32 = mybir.dt.float32
    BF16 = mybir.dt.bfloat16

    c1x = float(f32(1.0) / (f32(6.0) * f32(isc)))
    c1h = float(f32(1.0) / (f32(6.0) * f32(hsc)))
    inv2688 = float(f32(1.0) / f32(2688.0))
    RG = [list(range(NCORES))]

    nc = bacc.Bacc("TRN2", target_bir_lowering=False, debug=False,
                   num_devices=NCORES)
    x_sh = nc.dram_tensor("x_sh", [BSH, D_IN], FP32, kind="ExternalInput").ap()
    w1_sh = nc.dram_tensor("w1_sh", [HSH, D_IN], FP32, kind="ExternalInput").ap()
    w2_sh = nc.dram_tensor("w2_sh", [D_OUT, HSH], FP32, kind="ExternalInput").ap()
    out_sh = nc.dram_tensor("out_sh", [BSH, D_OUT], FP32, kind="ExternalOutput").ap()

    with tile.TileContext(nc) as tc, ExitStack() as top:
        dram = top.enter_context(tc.tile_pool(name="dram", bufs=1, space="DRAM"))
        amax_stage = dram.tile([128, 2], FP32, tag="amax_stage", name="amax_stage")
        s1loc = dram.tile([1, 1], FP32, tag="s1loc", name="s1loc")
        s2loc = dram.tile([1, 1], FP32, tag="s2loc", name="s2loc")
        s1sh = dram.tile([1, 1], FP32, tag="s1sh", name="s1sh", addr_space="Shared")
        s2sh = dram.tile([1, 1], FP32, tag="s2sh", name="s2sh", addr_space="Shared")
        xq_loc = dram.tile([BSH, D_IN], BF16, tag="xq_loc", name="xq_loc")
        xqT_loc = dram.tile([D_IN, BSH], BF16, tag="xqT_loc", name="xqT_loc")
        xqT_full = dram.tile([NCORES * D_IN, BSH], BF16, tag="xqT_full",
                             name="xqT_full", addr_space="Shared")
        w1q = dram.tile([HSH, D_IN], BF16, tag="w1q", name="w1q")
        w2q = dram.tile([D_OUT, HSH], BF16, tag="w2q", name="w2q")
        hq = dram.tile([B, HSH], BF16, tag="hq", name="hq")
        parts = [dram.tile([RSROWS, D_OUT], BF16, name=f"part{c}", tag=f"part{c}")
                 for c in range(RSCH)]
        rsouts = [dram.tile([RSOUT, D_OUT], BF16, name=f"rsout{c}",
                            tag=f"rsout{c}")
                  for c in range(RSCH)]

        singles = top.enter_context(tc.tile_pool(name="singles", bufs=1))
        biases = {}
        for nm, val in [("ch", C_HALF), ("nch", -C_HALF),
                        ("c1", C_1), ("nc1b", -C_1B),
                        ("c2", C_2), ("nc2b", -C_2B)]:
            bt = singles.tile([128, 1], FP32, tag=f"bias_{nm}", name=f"bias_{nm}")
            nc.vector.memset(bt[:], val)
            biases[nm] = bt

        # ================= Phase 0 =================
        # deep-buffered quant pipelines get the whole SBUF; w1T takes the
        # region over afterwards for its transposes + phase 1.
        with tc.tile_pool(name="p0src", bufs=4) as p0src, \
             tc.tile_pool(name="p0f", bufs=4) as p0f, \
             tc.tile_pool(name="p0b", bufs=4) as p0b, \
             tc.tile_pool(name="p0n", bufs=4) as p0n:
            acc1 = singles.tile([128, 1], FP32, tag="acc1", name="acc1")

            def w1_amax_chunk(i):
                # one [128, 1024] quarter-row-block of w1 -> running |.|max
                wt = p0src.tile([128, 1024], FP32, tag="wamax", name="wamax",
                                bufs=3)
                nc.scalar.dma_start(
                    wt[:], w1_sh[(i // 4) * 128:(i // 4 + 1) * 128,
                                 (i % 4) * 1024:(i % 4 + 1) * 1024])
                am = p0n.tile([128, 1], FP32, tag="am_w", name="am_w")
                nc.vector.tensor_reduce(am[:], wt[:],
                                        axis=mybir.AxisListType.X,
                                        op=OP.max, apply_absolute_value=True)
                if i == 0:
                    nc.vector.tensor_copy(acc1[:], am[:])
                else:
                    nc.vector.tensor_tensor(acc1[:], acc1[:], am[:], OP.max)

            def x_quant_tile(i, c):
                sl = slice(c * 1024, (c + 1) * 1024)
                xt = p0src.tile([128, 1024], FP32, tag="xt", name="xt")
                nc.scalar.dma_start(xt[:], x_sh[i * 128:(i + 1) * 128, sl])
                xo = p0src.tile([128, 1024], BF16, tag="xo", name="xo")
                _emit_quant(nc, mybir, p0f, p0b, p0n, biases,
                            xt[:], xo[:], c1x, float(isc), 1024)
                nc.sync.dma_start(xq_loc[i * 128:(i + 1) * 128, sl], xo[:])

            # ---- interleave the w1-amax stream with x-quant (col-major) ----
            for i in range(16):
                for u in range(4):
                    w1_amax_chunk(4 * i + u)
                if i % 2 == 1:
                    x_quant_tile(i // 2, 0)
            # finish w1-amax reduction -> AR1 (gpsimd ring slot #1)
            nc.sync.dma_start(amax_stage[:, 0:1], acc1[:])
            rowv1 = singles.tile([1, 128], FP32, tag="rowv1", name="rowv1")
            nc.sync.dma_start(
                rowv1[:], amax_stage[:, 0:1].rearrange("p c -> (p c)").unsqueeze(0))
            red1 = singles.tile([1, 1], FP32, tag="red1", name="red1")
            nc.vector.tensor_reduce(red1[:], rowv1[:],
                                    axis=mybir.AxisListType.X, op=OP.max)
            nc.sync.dma_start(s1loc[:], red1[:])
            nc.gpsimd.collective_compute(
                "AllReduce", OP.max, replica_groups=RG,
                ins=[s1loc[:].opt()], outs=[s1sh[:].opt()])
            sam1 = singles.tile([128, 1], FP32, tag="sam1", name="sam1")
            ap1 = s1sh[:]
            nc.gpsimd.dma_start(sam1[:], bass.AP(
                tensor=ap1.tensor, offset=ap1.offset,
                ap=[[0, 128]] + list(ap1.ap)[1:]))

            # ---- rest of x-quant, col-major; xqT transposes chase each
            # finished column-chunk ----
            def xqT_transpose(k):
                xtt = p0src.tile([128, BSH], BF16, tag="xtt", name="xtt",
                                 bufs=2)
                nc.sync.dma_start(xtt[:], xq_loc[:, k * 128:(k + 1) * 128],
                                  transpose=True)
                nc.sync.dma_start(xqT_loc[k * 128:(k + 1) * 128, :], xtt[:])

            for k in range(8):           # col-chunk 0 finished above
                xqT_transpose(k)
            for c in range(1, 4):
                for i in range(8):
                    x_quant_tile(i, c)
                for k in range(8 * c, 8 * c + 8):
                    xqT_transpose(k)
            nc.gpsimd.collective_compute(
                "AllGather", OP.bypass, replica_groups=RG,
                ins=[xqT_loc[:].opt()], outs=[xqT_full[:].opt()])

            # ---- w1 scale scalars (AR1 has long landed) ----
            tsw1 = singles.tile([128, 1], FP32, tag="tsw1", name="tsw1")
            nc.vector.tensor_scalar(tsw1[:], sam1[:], inv2688, None, OP.mult)
            dw1 = singles.tile([128, 1], FP32, tag="dw1", name="dw1")
            nc.vector.tensor_scalar(dw1[:], tsw1[:], 6.0, None, OP.mult)
            rdw1 = singles.tile([128, 1], FP32, tag="rdw1", name="rdw1")
            nc.vector.reciprocal(rdw1[:], dw1[:])
            s_h = singles.tile([128, 1], FP32, tag="s_h", name="s_h")
            nc.vector.tensor_scalar(s_h[:], tsw1[:], float(isc), None, OP.mult)

            # ---- quantize w1 (col-major; w1q lands in DRAM) ----
            for c in range(4):
                sl = slice(c * 1024, (c + 1) * 1024)
                for j in range(HSH // 128):
                    wt = p0src.tile([128, 1024], FP32, tag="xt", name="wt")
                    nc.scalar.dma_start(wt[:], w1_sh[j * 128:(j + 1) * 128, sl])
                    wo = p0src.tile([128, 1024], BF16, tag="xo", name="wo")
                    _emit_quant(nc, mybir, p0f, p0b, p0n, biases,
                                wt[:], wo[:], rdw1[:], tsw1[:], 1024)
                    nc.sync.dma_start(w1q[j * 128:(j + 1) * 128, sl], wo[:])

        # w1T region takes over the freed quant scratch; its transposes
        # run as soon as the w1q stores land.
        w1T_cm = tc.tile_pool(name="w1T", bufs=1)
        w1T_pool = w1T_cm.__enter__()
        w1T = w1T_pool.tile([128, NK1, HSH], BF16, tag="w1T", name="w1T")
        for k in range(NK1):
            nc.sync.dma_start(w1T[:, k, :], w1q[:, k * 128:(k + 1) * 128],
                              transpose=True)

        # ================= Phase 1 =================
        with tc.tile_pool(name="xb", bufs=2) as xb_pool, \
             tc.tile_pool(name="q1f", bufs=2) as q1f, \
             tc.tile_pool(name="q1b", bufs=2) as q1b, \
             tc.tile_pool(name="q1n", bufs=2) as q1n, \
             tc.tile_pool(name="w2s", bufs=2) as w2s, \
             tc.tile_pool(name="ps1", bufs=8, space="PSUM") as ps1:
            acc2 = singles.tile([128, 1], FP32, tag="acc2", name="acc2")
            tsw2 = singles.tile([128, 1], FP32, tag="tsw2", name="tsw2")
            rdw2 = singles.tile([128, 1], FP32, tag="rdw2", name="rdw2")
            s_o = singles.tile([128, 1], FP32, tag="s_o", name="s_o")

            # ---- main phase-1 loop; w2 amax / AR2 / half-quant mixed in ----
            for t in range(NBT):
                g0 = t * 128
                ci, off = divmod(g0, BSH)
                xb = xb_pool.tile([128, NK1, 128], BF16, tag="xb", name="xb")
                nc.sync.dma_start(
                    xb[:],
                    xqT_full[ci * D_IN:(ci + 1) * D_IN, off:off + 128]
                    .rearrange("(k p) c -> p k c", p=128))
                pss = [ps1.tile([128, 512], FP32, name="ps", tag="ps")
                       for _ in range(4)]
                for k in range(NK1):
                    for n in range(4):
                        nc.tensor.matmul(
                            pss[n][:], lhsT=xb[:, k, :],
                            rhs=w1T[:, k, n * 512:(n + 1) * 512],
                            start=(k == 0), stop=(k == NK1 - 1))
                for half in range(2):
                    g = q1f.tile([128, 1024], FP32, tag="q_g", name="q_g")
                    ho = q1b.tile([128, 1024], BF16, tag="q_ho", name="q_ho")
                    for n2 in range(2):
                        nc.scalar.activation(
                            g[:, n2 * 512:(n2 + 1) * 512],
                            pss[half * 2 + n2][:], AF.Gelu, scale=s_h[:])
                    _emit_quant(nc, mybir, q1f, q1b, q1n, biases,
                                g[:], ho[:], c1h, float(hsc), 1024,
                                signed=False)
                    nc.sync.dma_start(
                        hq[g0:g0 + 128, half * 1024:(half + 1) * 1024], ho[:])
                if t < 16:
                    # w2 amax stream: 4 x [128, 512] chunks per b-tile
                    for u in range(4):
                        i2 = 4 * t + u
                        wt3 = w2s.tile([128, 512], FP32, tag="wt2",
                                       name="wt3")
                        nc.scalar.dma_start(
                            wt3[:],
                            w2_sh[(i2 // 4) * 128:(i2 // 4 + 1) * 128,
                                  (i2 % 4) * 512:(i2 % 4 + 1) * 512])
                        am2 = q1n.tile([128, 1], FP32, tag="am_w2", name="am_w2")
                        nc.vector.tensor_reduce(am2[:], wt3[:],
                                                axis=mybir.AxisListType.X,
                                                op=OP.max,
                                                apply_absolute_value=True)
                        if i2 == 0:
                            nc.vector.tensor_copy(acc2[:], am2[:])
                        else:
                            nc.vector.tensor_tensor(acc2[:], acc2[:], am2[:],
                                                    OP.max)
                elif t == 16:
                    # all 64 w2-amax chunks are in; AR2 + scale scalars
                    nc.sync.dma_start(amax_stage[:, 1:2], acc2[:])
                    rowv2 = singles.tile([1, 128], FP32, tag="rowv2",
                                         name="rowv2")
                    nc.sync.dma_start(
                        rowv2[:],
                        amax_stage[:, 1:2].rearrange("p c -> (p c)").unsqueeze(0))
                    red2 = singles.tile([1, 1], FP32, tag="red2", name="red2")
                    nc.vector.tensor_reduce(red2[:], rowv2[:],
                                            axis=mybir.AxisListType.X, op=OP.max)
                    nc.sync.dma_start(s2loc[:], red2[:])
                    nc.gpsimd.collective_compute(
                        "AllReduce", OP.max, replica_groups=RG,
                        ins=[s2loc[:].opt()], outs=[s2sh[:].opt()])
                    sam2 = singles.tile([128, 1], FP32, tag="sam2", name="sam2")
                    ap2 = s2sh[:]
                    nc.gpsimd.dma_start(sam2[:], bass.AP(
                        tensor=ap2.tensor, offset=ap2.offset,
                        ap=[[0, 128]] + list(ap2.ap)[1:]))
                    nc.vector.tensor_scalar(tsw2[:], sam2[:], inv2688, None,
                                            OP.mult)
                    dw2 = singles.tile([128, 1], FP32, tag="dw2", name="dw2")
                    nc.vector.tensor_scalar(dw2[:], tsw2[:], 6.0, None, OP.mult)
                    nc.vector.reciprocal(rdw2[:], dw2[:])
                    nc.vector.tensor_scalar(s_o[:], tsw2[:], float(hsc), None,
                                            OP.mult)
                elif 17 <= t < 49:
                    # quantize only w2 rows 0..2047 here (2 chunks/b-tile);
                    # rows 2048..4095 quantize during phase-2 pass A.
                    for u in range(2):
                        i2 = 2 * (t - 17) + u
                        wi, cc = divmod(i2, 4)
                        sl = slice(cc * 512, (cc + 1) * 512)
                        wt2 = w2s.tile([128, 512], FP32, tag="wt2",
                                       name="wt2")
                        nc.scalar.dma_start(
                            wt2[:], w2_sh[wi * 128:(wi + 1) * 128, sl])
                        wo2 = w2s.tile([128, 512], BF16, tag="wo2",
                                       name="wo2")
                        _emit_quant(nc, mybir, q1f, q1b, q1n, biases,
                                    wt2[:], wo2[:], rdw2[:], tsw2[:],
                                    512)
                        nc.sync.dma_start(
                            w2q[wi * 128:(wi + 1) * 128, sl], wo2[:])

        # ================= Phase 2 =================
        w1T_cm.__exit__(None, None, None)
        with tc.tile_pool(name="w2Ta", bufs=1) as w2Ta_pool, \
             tc.tile_pool(name="w2Tb", bufs=1) as w2Tb_pool, \
             tc.tile_pool(name="hT", bufs=2) as hT_pool, \
             tc.tile_pool(name="osb", bufs=2) as osb, \
             tc.tile_pool(name="q2f", bufs=3) as q2f, \
             tc.tile_pool(name="q2b", bufs=2) as q2b, \
             tc.tile_pool(name="q2n", bufs=3) as q2n, \
             tc.tile_pool(name="w2s2", bufs=2) as w2s2, \
             tc.tile_pool(name="ps2", bufs=8, space="PSUM") as ps2:
            w2Ta = w2Ta_pool.tile([128, NK2, 2048], BF16, tag="w2Ta",
                                  name="w2Ta")
            w2Tb = w2Tb_pool.tile([128, NK2, 2048], BF16, tag="w2Tb",
                                  name="w2Tb")
            for k in range(NK2):
                nc.sync.dma_start(w2Ta[:, k, :],
                                  w2q[0:2048, k * 128:(k + 1) * 128],
                                  transpose=True)

            def p2_pass(colh, w2T):
                for sb in range(NSB):
                    r0 = sb * SB
                    hT = hT_pool.tile([128, NK2, SB], BF16, tag="hT",
                                      name="hT")
                    for k in range(NK2):
                        nc.sync.dma_start(
                            hT[:, k, :],
                            hq[r0:r0 + SB, k * 128:(k + 1) * 128],
                            transpose=True)
                    for b in range(SB // 128):
                        row = r0 + b * 128
                        c = row // RSROWS
                        crow = row % RSROWS
                        pss = [ps2.tile([128, 512], FP32, name="ps2",
                                        tag="ps2")
                               for _ in range(4)]
                        for k in range(NK2):
                            for n in range(4):
                                nc.tensor.matmul(
                                    pss[n][:],
                                    lhsT=hT[:, k, b * 128:(b + 1) * 128],
                                    rhs=w2T[:, k, n * 512:(n + 1) * 512],
                                    start=(k == 0), stop=(k == NK2 - 1))
                        ot = osb.tile([128, 2048], BF16, tag="ot", name="ot")
                        for n in range(4):
                            nc.scalar.activation(ot[:, n * 512:(n + 1) * 512],
                                                 pss[n][:], AF.Copy,
                                                 scale=s_o[:])
                        nc.sync.dma_start(
                            parts[c][crow:crow + 128,
                                     colh * 2048:(colh + 1) * 2048], ot[:])
                    if colh == 0:
                        # pass A: idle Vector quantizes w2 rows 2048..4095
                        for u in range(4):
                            i2 = 4 * sb + u
                            wi, cc = divmod(i2, 4)
                            wi += 16
                            sl = slice(cc * 512, (cc + 1) * 512)
                            wt2 = w2s2.tile([128, 512], FP32, tag="wt2b",
                                            name="wt2b")
                            nc.scalar.dma_start(
                                wt2[:], w2_sh[wi * 128:(wi + 1) * 128, sl])
                            wo2 = w2s2.tile([128, 512], BF16, tag="wo2b",
                                            name="wo2b")
                            _emit_quant(nc, mybir, q2f, q2b, q2n, biases,
                                        wt2[:], wo2[:], rdw2[:], tsw2[:],
                                        512)
                            nc.sync.dma_start(
                                w2q[wi * 128:(wi + 1) * 128, sl], wo2[:])
                    else:
                        # pass B: chunk sb rows are complete -> ReduceScatter
                        nc.gpsimd.collective_compute(
                            "ReduceScatter", OP.add, replica_groups=RG,
                            ins=[parts[sb][:].opt()],
                            outs=[rsouts[sb][:].opt()])

                if colh == 0:
                    for k in range(NK2):
                        nc.sync.dma_start(
                            w2Tb[:, k, :],
                            w2q[2048:4096, k * 128:(k + 1) * 128],
                            transpose=True)

            p2_pass(0, w2Ta)
            p2_pass(1, w2Tb)
            # per-chunk f32 casts of the landed rs outputs (SWDGE cast-DMA);
            # chunk c's store fires as soon as its collective completes.
            for c in range(RSCH):
                nc.gpsimd.dma_start(out_sh[c * RSOUT:(c + 1) * RSOUT, :],
                                    rsouts[c][:])
    nc.compile()
    return nc


def _get_built(isc, hsc):
    key = (float(isc), float(hsc), USE_GP)
    if key not in _BUILT:
        _BUILT[key] = _build(float(isc), float(hsc))
    return _BUILT[key]


def run(x, w1, w2, input_scale, hidden_scale, trace=False):
    from concourse import bass_utils
    isc = float(np.asarray(input_scale).reshape(-1)[0])
    hsc = float(np.asarray(hidden_scale).reshape(-1)[0])
    nc = _get_built(isc, hsc)
    x = np.ascontiguousarray(x, dtype=np.float32)
    w1 = np.ascontiguousarray(w1, dtype=np.float32)
    w2 = np.ascontiguousarray(w2, dtype=np.float32)
    in_maps = []
    for c in range(NCORES):
        in_maps.append({
            "x_sh": x[c * BSH:(c + 1) * BSH, :],
            "w1_sh": np.ascontiguousarray(w1[c * HSH:(c + 1) * HSH, :]),
            "w2_sh": np.ascontiguousarray(w2[:, c * HSH:(c + 1) * HSH]),
        })
    res = bass_utils.run_bass_kernel_spmd(
        nc, in_maps, core_ids=list(range(NCORES)), trace=trace)
    out = np.empty((B, D_OUT), dtype=np.float32)
    for r in range(NCORES):
        o = res.results[r]["out_sh"]
        for c in range(RSCH):
            out[c * RSROWS + r * RSOUT:c * RSROWS + (r + 1) * RSOUT, :] = \
                o[c * RSOUT:(c + 1) * RSOUT, :]
    return out, res


def kernel(x, w1, w2, input_scale, hidden_scale):
    out, _ = run(x, w1, w2, input_scale, hidden_scale, trace=False)
    return out


# revision 19
# speedup vs baseline: 1.0275x; 1.0114x over previous
"""NVFP4-fake-quant MLP (x@w1.T -> gelu -> @w2.T) on 8 trn2 NeuronCores.

Sharding (megatron tensor-parallel on the hidden dim):
  core c holds w1 rows [c*2048:(c+1)*2048], w2 cols [c*2048:(c+1)*2048],
  and x rows [c*1024:(c+1)*1024] (for distributed x-quantization).

Exact quantization:
  per-16-block e4m3 scales via exponent-mask + magic-number RNE;
  fp4 e2m1 rounding via 3-region clamp + magic-round decomposition.
  e2m1_value * e4m3_blockscale has <= 6 mantissa bits -> stored EXACTLY in
  bf16, so the bf16 matmuls reproduce the f32 reference; per-tensor scales
  are folded into the PSUM->SBUF copies (gelu input scale / output scale).

Dataflow / overlap (v3):
  Prologue: w1-amax chunks + x-quant interleave from t=0; AR1 early; xqT
  transposes chase x-quant per column-chunk so the AllGather hides under
  w1-quant.  Quant pools are deep (bufs=4, no w1T reservation yet) so the
  ~19-op cross-engine DVE/ACT quant chain pipelines; w1T transposes +
  phase-1 pools take over the region afterwards.
  Phase 1: w2 amax for t<16, AR2 at t=16, then only the FIRST half of w2
  (rows 0..2047) quantizes at 2 chunks/b-tile -- Vector stays under the
  PE period, no PSUM-drain stalls.
  Phase 2 runs in two column-half passes: pass A (out cols 0..2047) needs
  only w2T-half-A, and its idle Vector quantizes w2 rows 2048..4095;
  pass B computes cols 2048..4095 and fires one bf16 ReduceScatter per
  512-row chunk (16 total, spread evenly), each chunk's f32 cast-store
  issued by gpsimd DMA as soon as its collective lands.  Partials are
  bf16 throughout: half the HBM traffic and half the RS wire bytes.
"""
import os
import sys
import numpy as np

if "/opt/trn_rl_repo" not in sys.path:
    sys.path.insert(0, "/opt/trn_rl_repo")

f32 = np.float32

B, D_IN, HID, D_OUT = 8192, 4096, 16384, 4096
NCORES = 8
BSH = B // NCORES          # 1024 x-rows quantized per core
HSH = HID // NCORES        # 2048 hidden units per core
SB = 512                   # phase-2 transpose-load superblock rows
NSB = B // SB
NBT = B // 128             # 64 b-tiles
RSCH = 16                  # reduce-scatter chunks
RSROWS = B // RSCH         # 512 rows per RS chunk
RSOUT = RSROWS // NCORES   # 64 rows per core per chunk
NK1 = D_IN // 128          # 32 k-tiles, first matmul
NK2 = HSH // 128           # 16 k-tiles, second matmul

# magic round-to-nearest-even constants (f32-exact)
C_HALF = float(f32(1.5 * 2 ** 22))       # grid 0.5
C_1 = float(f32(1.5 * 2 ** 23))          # grid 1
C_1B = float(f32(1.5 * 2 ** 23 + 2.0))   # C_1 + 2
C_2 = float(f32(1.5 * 2 ** 24))          # grid 2
C_2B = float(f32(1.5 * 2 ** 24 + 4.0))   # C_2 + 4
E4M3_MAGIC = float(f32(1.5 * 2 ** 20))   # * 2^e -> magic const for step 2^(e-3)
EXPMASK = 0x7F800000
SIGNMASK = 0x80000000
ONEBITS = 0x3F800000

_BUILT = {}
USE_GP = os.environ.get("KQ_USE_GPSIMD", "0") == "1"  # Pool ALU ~15x slower than DVE; keep off


def _emit_quant(nc, mybir, pf, pb, pn, biases, src, out, c1, effmul, W,
                signed=True):
    """Quantize src [128, W] f32 (SBUF) -> out [128, W] bf16 = sign*e2m1*bscale.

    c1: 1/(6*tensor_scale)  (float imm or [128,1] AP)
    effmul: tensor_scale    (float imm or [128,1] AP)
    biases: dict of [128,1] f32 bias tiles for the ACT magic rounds.

    signed=False is valid when every negative src value is guaranteed to
    quantize to 0 (gelu outputs: |neg| <= 0.17 < 0.375 <= 0.25*eff_min);
    the signed r then flows through the clamps/magic-rounds to exact 0s,
    saving the abs / sign-extract / sign-multiply ops.

"""
    OP = mybir.AluOpType
    AF = mybir.ActivationFunctionType
    U32 = mybir.dt.uint32
    FP32 = mybir.dt.float32
    BF16 = mybir.dt.bfloat16
    NB = W // 16
    ev = nc.gpsimd if USE_GP else nc.vector

    if signed:
        absv = pf.tile([128, W], FP32, tag="q_absv", name="q_absv")
        nc.scalar.activation(absv[:], src, AF.Abs)
    else:
        absv = None
    amax = pn.tile([128, NB], FP32, tag="q_amax", name="q_amax")
    nc.vector.tensor_reduce(
        amax[:],
        (absv[:] if signed else src).rearrange("p (nb b) -> p nb b", b=16),
        axis=mybir.AxisListType.X, op=OP.max,
        apply_absolute_value=(None if signed else True))
    vq = pn.tile([128, NB], FP32, tag="q_vq", name="q_vq")
    nc.vector.tensor_scalar(vq[:], amax[:], c1, None, OP.mult)
    scq = pn.tile([128, NB], FP32, tag="q_scq", name="q_scq")
    nc.vector.tensor_scalar(scq[:].bitcast(U32), vq[:].bitcast(U32),
                            EXPMASK, None, OP.bitwise_and)
    cb = pn.tile([128, NB], FP32, tag="q_cb", name="q_cb")
    nc.vector.tensor_scalar(cb[:], scq[:], E4M3_MAGIC, None, OP.mult)
    t4 = pn.tile([128, NB], FP32, tag="q_t4", name="q_t4")
    nc.vector.tensor_tensor(t4[:], vq[:], cb[:], OP.add)
    bs = pn.tile([128, NB], FP32, tag="q_bs", name="q_bs")
    nc.vector.tensor_tensor(bs[:], t4[:], cb[:], OP.subtract)
    bs16 = pn.tile([128, NB], BF16, tag="q_bs16", name="q_bs16")
    nc.vector.tensor_scalar(bs16[:], bs[:], 2.0 ** -6, None, OP.max)
    eff = pn.tile([128, NB], FP32, tag="q_eff", name="q_eff")
    nc.vector.tensor_scalar(eff[:], bs[:], 2.0 ** -6, effmul, OP.max, OP.mult)
    rec = pn.tile([128, NB], FP32, tag="q_rec", name="q_rec")
    nc.vector.reciprocal(rec[:], eff[:])
    r = pf.tile([128, W], FP32, tag="q_r", name="q_r")
    nc.vector.tensor_tensor(
        r[:].rearrange("p (nb b) -> p nb b", b=16),
        (absv[:] if signed else src).rearrange("p (nb b) -> p nb b", b=16),
        rec[:, :, None].to_broadcast([128, NB, 16]), OP.mult)
    m1 = pf.tile([128, W], FP32, tag="q_absv", name="q_m1")   # reuse absv slots
    nc.vector.tensor_scalar(m1[:], r[:], 2.0, None, OP.min)
    m3 = pf.tile([128, W], FP32, tag="q_m23", name="q_m3")
    ev.tensor_scalar(m3[:], r[:], 4.0, 6.0, OP.max, OP.min)
    nc.scalar.activation(m3[:], m3[:], AF.Identity, bias=biases["c2"][:])
    s3 = pb.tile([128, W], BF16, tag="q_s3", name="q_s3")
    nc.scalar.activation(s3[:], m3[:], AF.Identity, bias=biases["nc2b"][:])
    m2 = pf.tile([128, W], FP32, tag="q_m23", name="q_m2")
    ev.tensor_scalar(m2[:], r[:], 2.0, 4.0, OP.max, OP.min)
    nc.scalar.activation(m2[:], m2[:], AF.Identity, bias=biases["c1"][:])
    s2 = pb.tile([128, W], BF16, tag="q_s2", name="q_s2")
    nc.scalar.activation(s2[:], m2[:], AF.Identity, bias=biases["nc1b"][:])
    nc.scalar.activation(m1[:], m1[:], AF.Identity, bias=biases["ch"][:])
    s1 = pb.tile([128, W], BF16, tag="q_s1", name="q_s1")
    nc.scalar.activation(s1[:], m1[:], AF.Identity, bias=biases["nch"][:])
    q12 = pb.tile([128, W], BF16, tag="q_s1", name="q_q12")
    ev.tensor_tensor(q12[:], s1[:], s2[:], OP.add)
    qq = pb.tile([128, W], BF16, tag="q_s2", name="q_qq")
    ev.tensor_tensor(qq[:], q12[:], s3[:], OP.add)
    if signed:
        qs = pb.tile([128, W], BF16, tag="q_s1", name="q_qs")
        nc.vector.tensor_tensor(
            qs[:].rearrange("p (nb b) -> p nb b", b=16),
            qq[:].rearrange("p (nb b) -> p nb b", b=16),
            bs16[:, :, None].to_broadcast([128, NB, 16]), OP.mult)
        sgn = pf.tile([128, W], FP32, tag="q_r", name="q_sgn")  # reuse r slots
        nc.vector.tensor_scalar(sgn[:].bitcast(U32), src.bitcast(U32),
                                SIGNMASK, ONEBITS,
                                OP.bitwise_and, OP.bitwise_or)
        nc.vector.tensor_tensor(out, qs[:], sgn[:], OP.mult)
    else:
        nc.vector.tensor_tensor(
            out.rearrange("p (nb b) -> p nb b", b=16),
            qq[:].rearrange("p (nb b) -> p nb b", b=16),
            bs16[:, :, None].to_broadcast([128, NB, 16]), OP.mult)


def _build(isc, hsc):
    from contextlib import ExitStack
    import concourse.bass as bass
    import concourse.tile as tile
    from concourse import bacc, mybir

    OP = mybir.AluOpType
    AF = mybir.ActivationFunctionType
    FP32 = mybir.dt.float32
    BF16 = mybir.dt.bfloat16

    c1x = float(f32(1.0) / (f32(6.0) * f32(isc)))
    c1h = float(f32(1.0) / (f32(6.0) * f32(hsc)))
    inv2688 = float(f32(1.0) / f32(2688.0))
    RG = [list(range(NCORES))]

    nc = bacc.Bacc("TRN2", target_bir_lowering=False, debug=False,
                   num_devices=NCORES)
    x_sh = nc.dram_tensor("x_sh", [BSH, D_IN], FP32, kind="ExternalInput").ap()
    w1_sh = nc.dram_tensor("w1_sh", [HSH, D_IN], FP32, kind="ExternalInput").ap()
    w2_sh = nc.dram_tensor("w2_sh", [D_OUT, HSH], FP32, kind="ExternalInput").ap()
    out_sh = nc.dram_tensor("out_sh", [BSH, D_OUT], FP32, kind="ExternalOutput").ap()

    with tile.TileContext(nc) as tc, ExitStack() as top:
        dram = top.enter_context(tc.tile_pool(name="dram", bufs=1, space="DRAM"))
        amax_stage = dram.tile([128, 2], FP32, tag="amax_stage", name="amax_stage")
        s1loc = dram.tile([1, 1], FP32, tag="s1loc", name="s1loc")
        s2loc = dram.tile([1, 1], FP32, tag="s2loc", name="s2loc")
        s1sh = dram.tile([1, 1], FP32, tag="s1sh", name="s1sh", addr_space="Shared")
        s2sh = dram.tile([1, 1], FP32, tag="s2sh", name="s2sh", addr_space="Shared")
        xq_loc = dram.tile([BSH, D_IN], BF16, tag="xq_loc", name="xq_loc")
        xqT_locs = [dram.tile([D_IN, BSH // 2], BF16, tag=f"xqT_loc{h}",
                              name=f"xqT_loc{h}") for h in range(2)]
        xqT_fulls = [dram.tile([NCORES * D_IN, BSH // 2], BF16,
                               tag=f"xqT_full{h}", name=f"xqT_full{h}",
                               addr_space="Shared") for h in range(2)]
        w1q = dram.tile([HSH, D_IN], BF16, tag="w1q", name="w1q")
        w2q = dram.tile([D_OUT, HSH], BF16, tag="w2q", name="w2q")
        hq = dram.tile([B, HSH], BF16, tag="hq", name="hq")
        parts = [dram.tile([RSROWS, D_OUT], BF16, name=f"part{c}", tag=f"part{c}")
                 for c in range(RSCH)]
        rsouts = [dram.tile([RSOUT, D_OUT], BF16, name=f"rsout{c}",
                            tag=f"rsout{c}")
                  for c in range(RSCH)]

        singles = top.enter_context(tc.tile_pool(name="singles", bufs=1))
        biases = {}
        for nm, val in [("ch", C_HALF), ("nch", -C_HALF),
                        ("c1", C_1), ("nc1b", -C_1B),
                        ("c2", C_2), ("nc2b", -C_2B)]:
            bt = singles.tile([128, 1], FP32, tag=f"bias_{nm}", name=f"bias_{nm}")
            nc.vector.memset(bt[:], val)
            biases[nm] = bt

        # ================= Phase 0 =================
        # deep-buffered quant pipelines get the whole SBUF; w1T takes the
        # region over afterwards for its transposes + phase 1.
        with tc.tile_pool(name="p0src", bufs=4) as p0src, \
             tc.tile_pool(name="p0f", bufs=4) as p0f, \
             tc.tile_pool(name="p0b", bufs=4) as p0b, \
             tc.tile_pool(name="p0n", bufs=4) as p0n:
            acc1 = singles.tile([128, 1], FP32, tag="acc1", name="acc1")

            def w1_amax_chunk(i):
                # one [128, 1024] quarter-row-block of w1 -> running |.|max
                wt = p0src.tile([128, 1024], FP32, tag="wamax", name="wamax",
                                bufs=3)
                nc.scalar.dma_start(
                    wt[:], w1_sh[(i // 4) * 128:(i // 4 + 1) * 128,
                                 (i % 4) * 1024:(i % 4 + 1) * 1024])
                am = p0n.tile([128, 1], FP32, tag="am_w", name="am_w")
                nc.vector.tensor_reduce(am[:], wt[:],
                                        axis=mybir.AxisListType.X,
                                        op=OP.max, apply_absolute_value=True)
                if i == 0:
                    nc.vector.tensor_copy(acc1[:], am[:])
                else:
                    nc.vector.tensor_tensor(acc1[:], acc1[:], am[:], OP.max)

            def x_quant_tile(i, c):
                sl = slice(c * 1024, (c + 1) * 1024)
                xt = p0src.tile([128, 1024], FP32, tag="xt", name="xt")
                nc.scalar.dma_start(xt[:], x_sh[i * 128:(i + 1) * 128, sl])
                xo = p0src.tile([128, 1024], BF16, tag="xo", name="xo")
                _emit_quant(nc, mybir, p0f, p0b, p0n, biases,
                            xt[:], xo[:], c1x, float(isc), 1024)
                nc.sync.dma_start(xq_loc[i * 128:(i + 1) * 128, sl], xo[:])

            # ---- interleave the w1-amax stream with x-quant (row-major,
            # batch-half 0) so its half-AllGather can fire mid-prologue ----
            for i in range(16):
                for u in range(4):
                    w1_amax_chunk(4 * i + u)
                x_quant_tile(i // 4, i % 4)
            # finish w1-amax reduction -> AR1 (gpsimd ring slot #1)
            nc.sync.dma_start(amax_stage[:, 0:1], acc1[:])
            rowv1 = singles.tile([1, 128], FP32, tag="rowv1", name="rowv1")
            nc.sync.dma_start(
                rowv1[:], amax_stage[:, 0:1].rearrange("p c -> (p c)").unsqueeze(0))
            red1 = singles.tile([1, 1], FP32, tag="red1", name="red1")
            nc.vector.tensor_reduce(red1[:], rowv1[:],
                                    axis=mybir.AxisListType.X, op=OP.max)
            nc.sync.dma_start(s1loc[:], red1[:])
            nc.gpsimd.collective_compute(
                "AllReduce", OP.max, replica_groups=RG,
                ins=[s1loc[:].opt()], outs=[s1sh[:].opt()])
            sam1 = singles.tile([128, 1], FP32, tag="sam1", name="sam1")
            ap1 = s1sh[:]
            nc.gpsimd.dma_start(sam1[:], bass.AP(
                tensor=ap1.tensor, offset=ap1.offset,
                ap=[[0, 128]] + list(ap1.ap)[1:]))

            # ---- per-batch-half xqT transposes + AllGathers: half 0's AG
            # fires at ~50% of x-quant, spreading its ring HBM traffic off
            # the w1-quant window ----
            def xqT_transpose(h, k):
                xtt = p0src.tile([128, BSH // 2], BF16, tag="xtt", name="xtt",
                                 bufs=2)
                nc.sync.dma_start(
                    xtt[:],
                    xq_loc[h * 512:(h + 1) * 512, k * 128:(k + 1) * 128],
                    transpose=True)
                nc.sync.dma_start(xqT_locs[h][k * 128:(k + 1) * 128, :],
                                  xtt[:])

            for k in range(NK1):         # batch-half 0 finished above
                xqT_transpose(0, k)
            nc.gpsimd.collective_compute(
                "AllGather", OP.bypass, replica_groups=RG,
                ins=[xqT_locs[0][:].opt()], outs=[xqT_fulls[0][:].opt()])
            for i in range(4, 8):
                for c in range(4):
                    x_quant_tile(i, c)
            for k in range(NK1):
                xqT_transpose(1, k)
            nc.gpsimd.collective_compute(
                "AllGather", OP.bypass, replica_groups=RG,
                ins=[xqT_locs[1][:].opt()], outs=[xqT_fulls[1][:].opt()])

            # ---- w1 scale scalars (AR1 has long landed) ----
            tsw1 = singles.tile([128, 1], FP32, tag="tsw1", name="tsw1")
            nc.vector.tensor_scalar(tsw1[:], sam1[:], inv2688, None, OP.mult)
            dw1 = singles.tile([128, 1], FP32, tag="dw1", name="dw1")
            nc.vector.tensor_scalar(dw1[:], tsw1[:], 6.0, None, OP.mult)
            rdw1 = singles.tile([128, 1], FP32, tag="rdw1", name="rdw1")
            nc.vector.reciprocal(rdw1[:], dw1[:])
            s_h = singles.tile([128, 1], FP32, tag="s_h", name="s_h")
            nc.vector.tensor_scalar(s_h[:], tsw1[:], float(isc), None, OP.mult)

            # ---- quantize w1 (col-major; w1q lands in DRAM) ----
            for c in range(4):
                sl = slice(c * 1024, (c + 1) * 1024)
                for j in range(HSH // 128):
                    wt = p0src.tile([128, 1024], FP32, tag="xt", name="wt")
                    nc.scalar.dma_start(wt[:], w1_sh[j * 128:(j + 1) * 128, sl])
                    wo = p0src.tile([128, 1024], BF16, tag="xo", name="wo")
                    _emit_quant(nc, mybir, p0f, p0b, p0n, biases,
                                wt[:], wo[:], rdw1[:], tsw1[:], 1024)
                    nc.sync.dma_start(w1q[j * 128:(j + 1) * 128, sl], wo[:])

        # w1T region takes over the freed quant scratch; its transposes
        # run as soon as the w1q stores land.
        w1T_cm = tc.tile_pool(name="w1T", bufs=1)
        w1T_pool = w1T_cm.__enter__()
        w1T = w1T_pool.tile([128, NK1, HSH], BF16, tag="w1T", name="w1T")
        for k in range(NK1):
            nc.sync.dma_start(w1T[:, k, :], w1q[:, k * 128:(k + 1) * 128],
                              transpose=True)

        # ================= Phase 1 =================
        with tc.tile_pool(name="xb", bufs=2) as xb_pool, \
             tc.tile_pool(name="q1f", bufs=2) as q1f, \
             tc.tile_pool(name="q1b", bufs=2) as q1b, \
             tc.tile_pool(name="q1n", bufs=2) as q1n, \
             tc.tile_pool(name="w2s", bufs=2) as w2s, \
             tc.tile_pool(name="ps1", bufs=8, space="PSUM") as ps1:
            acc2 = singles.tile([128, 1], FP32, tag="acc2", name="acc2")
            tsw2 = singles.tile([128, 1], FP32, tag="tsw2", name="tsw2")
            rdw2 = singles.tile([128, 1], FP32, tag="rdw2", name="rdw2")
            s_o = singles.tile([128, 1], FP32, tag="s_o", name="s_o")

            # ---- main phase-1 loop; w2 amax / AR2 / half-quant mixed in ----
            for t in range(NBT):
                g0 = t * 128
                ci, off = divmod(g0, BSH)
                hb, o2 = divmod(off, 512)
                xb = xb_pool.tile([128, NK1, 128], BF16, tag="xb", name="xb")
                nc.sync.dma_start(
                    xb[:],
                    xqT_fulls[hb][ci * D_IN:(ci + 1) * D_IN, o2:o2 + 128]
                    .rearrange("(k p) c -> p k c", p=128))
                pss = [ps1.tile([128, 512], FP32, name="ps", tag="ps")
                       for _ in range(4)]
                for k in range(NK1):
                    for n in range(4):
                        nc.tensor.matmul(
                            pss[n][:], lhsT=xb[:, k, :],
                            rhs=w1T[:, k, n * 512:(n + 1) * 512],
                            start=(k == 0), stop=(k == NK1 - 1))
                for half in range(2):
                    g = q1f.tile([128, 1024], FP32, tag="q_g", name="q_g")
                    ho = q1b.tile([128, 1024], BF16, tag="q_ho", name="q_ho")
                    for n2 in range(2):
                        nc.scalar.activation(
                            g[:, n2 * 512:(n2 + 1) * 512],
                            pss[half * 2 + n2][:], AF.Gelu, scale=s_h[:])
                    _emit_quant(nc, mybir, q1f, q1b, q1n, biases,
                                g[:], ho[:], c1h, float(hsc), 1024,
                                signed=False)
                    nc.sync.dma_start(
                        hq[g0:g0 + 128, half * 1024:(half + 1) * 1024], ho[:])
                if t < 16:
                    # w2 amax stream: 4 x [128, 512] chunks per b-tile
                    for u in range(4):
                        i2 = 4 * t + u
                        wt3 = w2s.tile([128, 512], FP32, tag="wt2",
                                       name="wt3")
                        nc.scalar.dma_start(
                            wt3[:],
                            w2_sh[(i2 // 4) * 128:(i2 // 4 + 1) * 128,
                                  (i2 % 4) * 512:(i2 % 4 + 1) * 512])
                        am2 = q1n.tile([128, 1], FP32, tag="am_w2", name="am_w2")
                        nc.vector.tensor_reduce(am2[:], wt3[:],
                                                axis=mybir.AxisListType.X,
                                                op=OP.max,
                                                apply_absolute_value=True)
                        if i2 == 0:
                            nc.vector.tensor_copy(acc2[:], am2[:])
                        else:
                            nc.vector.tensor_tensor(acc2[:], acc2[:], am2[:],
                                                    OP.max)
                elif t == 16:
                    # all 64 w2-amax chunks are in; AR2 + scale scalars
                    nc.sync.dma_start(amax_stage[:, 1:2], acc2[:])
                    rowv2 = singles.tile([1, 128], FP32, tag="rowv2",
                                         name="rowv2")
                    nc.sync.dma_start(
                        rowv2[:],
                        amax_stage[:, 1:2].rearrange("p c -> (p c)").unsqueeze(0))
                    red2 = singles.tile([1, 1], FP32, tag="red2", name="red2")
                    nc.vector.tensor_reduce(red2[:], rowv2[:],
                                            axis=mybir.AxisListType.X, op=OP.max)
                    nc.sync.dma_start(s2loc[:], red2[:])
                    nc.gpsimd.collective_compute(
                        "AllReduce", OP.max, replica_groups=RG,
                        ins=[s2loc[:].opt()], outs=[s2sh[:].opt()])
                    sam2 = singles.tile([128, 1], FP32, tag="sam2", name="sam2")
                    ap2 = s2sh[:]
                    nc.gpsimd.dma_start(sam2[:], bass.AP(
                        tensor=ap2.tensor, offset=ap2.offset,
                        ap=[[0, 128]] + list(ap2.ap)[1:]))
                    nc.vector.tensor_scalar(tsw2[:], sam2[:], inv2688, None,
                                            OP.mult)
                    dw2 = singles.tile([128, 1], FP32, tag="dw2", name="dw2")
                    nc.vector.tensor_scalar(dw2[:], tsw2[:], 6.0, None, OP.mult)
                    nc.vector.reciprocal(rdw2[:], dw2[:])
                    nc.vector.tensor_scalar(s_o[:], tsw2[:], float(hsc), None,
                                            OP.mult)
                elif 17 <= t < 49:
                    # quantize only w2 rows 0..2047 here (2 chunks/b-tile);
                    # rows 2048..4095 quantize during phase-2 pass A.
                    for u in range(2):
                        i2 = 2 * (t - 17) + u
                        wi, cc = divmod(i2, 4)
                        sl = slice(cc * 512, (cc + 1) * 512)
                        wt2 = w2s.tile([128, 512], FP32, tag="wt2",
                                       name="wt2")
                        nc.scalar.dma_start(
                            wt2[:], w2_sh[wi * 128:(wi + 1) * 128, sl])
                        wo2 = w2s.tile([128, 512], BF16, tag="wo2",
                                       name="wo2")
                        _emit_quant(nc, mybir, q1f, q1b, q1n, biases,
                                    wt2[:], wo2[:], rdw2[:], tsw2[:],
                                    512)
                        nc.sync.dma_start(
                            w2q[wi * 128:(wi + 1) * 128, sl], wo2[:])

        # ================= Phase 2 =================
        w1T_cm.__exit__(None, None, None)
        with tc.tile_pool(name="w2Ta", bufs=1) as w2Ta_pool, \
             tc.tile_pool(name="w2Tb", bufs=1) as w2Tb_pool, \
             tc.tile_pool(name="osb", bufs=2) as osb, \
             tc.tile_pool(name="ps2", bufs=8, space="PSUM") as ps2:
            w2Ta = w2Ta_pool.tile([128, NK2, 2048], BF16, tag="w2Ta",
                                  name="w2Ta")
            w2Tb = w2Tb_pool.tile([128, NK2, 2048], BF16, tag="w2Tb",
                                  name="w2Tb")
            for k in range(NK2):
                nc.sync.dma_start(w2Ta[:, k, :],
                                  w2q[0:2048, k * 128:(k + 1) * 128],
                                  transpose=True)

            def p2_pass(colh, w2T, hT_pool, sbr, quant_cb):
                for sb in range(B // sbr):
                    r0 = sb * sbr
                    hT = hT_pool.tile([128, NK2, sbr], BF16, tag="hT",
                                      name="hT")
                    for k in range(NK2):
                        nc.sync.dma_start(
                            hT[:, k, :],
                            hq[r0:r0 + sbr, k * 128:(k + 1) * 128],
                            transpose=True)
                    for b in range(sbr // 128):
                        row = r0 + b * 128
                        c = row // RSROWS
                        crow = row % RSROWS
                        pss = [ps2.tile([128, 512], FP32, name="ps2",
                                        tag="ps2")
                               for _ in range(4)]
                        for k in range(NK2):
                            for n in range(4):
                                nc.tensor.matmul(
                                    pss[n][:],
                                    lhsT=hT[:, k, b * 128:(b + 1) * 128],
                                    rhs=w2T[:, k, n * 512:(n + 1) * 512],
                                    start=(k == 0), stop=(k == NK2 - 1))
                        ot = osb.tile([128, 2048], BF16, tag="ot", name="ot")
                        for n in range(4):
                            nc.scalar.activation(ot[:, n * 512:(n + 1) * 512],
                                                 pss[n][:], AF.Copy,
                                                 scale=s_o[:])
                        nc.sync.dma_start(
                            parts[c][crow:crow + 128,
                                     colh * 2048:(colh + 1) * 2048], ot[:])
                    quant_cb(sb)

            with tc.tile_pool(name="hTa", bufs=2) as hTa_pool, \
                 tc.tile_pool(name="q2f", bufs=3) as q2f, \
                 tc.tile_pool(name="q2b", bufs=2) as q2b, \
                 tc.tile_pool(name="q2n", bufs=3) as q2n, \
                 tc.tile_pool(name="w2s2", bufs=2) as w2s2:
                def passa_cb(sb):
                    # pass A: idle Vector quantizes w2 rows 2048..4095
                    for u in range(4):
                        i2 = 4 * sb + u
                        wi, cc = divmod(i2, 4)
                        wi += 16
                        sl = slice(cc * 512, (cc + 1) * 512)
                        wt2 = w2s2.tile([128, 512], FP32, tag="wt2b",
                                        name="wt2b")
                        nc.scalar.dma_start(
                            wt2[:], w2_sh[wi * 128:(wi + 1) * 128, sl])
                        wo2 = w2s2.tile([128, 512], BF16, tag="wo2b",
                                        name="wo2b")
                        _emit_quant(nc, mybir, q2f, q2b, q2n, biases,
                                    wt2[:], wo2[:], rdw2[:], tsw2[:], 512)
                        nc.sync.dma_start(
                            w2q[wi * 128:(wi + 1) * 128, sl], wo2[:])

                p2_pass(0, w2Ta, hTa_pool, 512, passa_cb)
                for k in range(NK2):
                    nc.sync.dma_start(
                        w2Tb[:, k, :],
                        w2q[2048:4096, k * 128:(k + 1) * 128],
                        transpose=True)

            # pass B gets the quant pools' region back as a deep 1024-row
            # hT double-buffer, so RS bursts can't starve the transposes.
            with tc.tile_pool(name="hTb", bufs=2) as hTb_pool:
                def passb_cb(sb):
                    for cc in (2 * sb, 2 * sb + 1):
                        nc.gpsimd.collective_compute(
                            "ReduceScatter", OP.add, replica_groups=RG,
                            ins=[parts[cc][:].opt()],
                            outs=[rsouts[cc][:].opt()])

                p2_pass(1, w2Tb, hTb_pool, 1024, passb_cb)
            # per-chunk f32 casts of the landed rs outputs (SWDGE cast-DMA);
            # chunk c's store fires as soon as its collective completes.
            for c in range(RSCH):
                nc.gpsimd.dma_start(out_sh[c * RSOUT:(c + 1) * RSOUT, :],
                                    rsouts[c][:])
    nc.compile()
    return nc


def _get_built(isc, hsc):
    key = (float(isc), float(hsc), USE_GP)
    if key not in _BUILT:
        _BUILT[key] = _build(float(isc), float(hsc))
    return _BUILT[key]


def run(x, w1, w2, input_scale, hidden_scale, trace=False):
    from concourse import bass_utils
    isc = float(np.asarray(input_scale).reshape(-1)[0])
    hsc = float(np.asarray(hidden_scale).reshape(-1)[0])
    nc = _get_built(isc, hsc)
    x = np.ascontiguousarray(x, dtype=np.float32)
    w1 = np.ascontiguousarray(w1, dtype=np.float32)
    w2 = np.ascontiguousarray(w2, dtype=np.float32)
    in_maps = []
    for c in range(NCORES):
        in_maps.append({
            "x_sh": x[c * BSH:(c + 1) * BSH, :],
            "w1_sh": np.ascontiguousarray(w1[c * HSH:(c + 1) * HSH, :]),
            "w2_sh": np.ascontiguousarray(w2[:, c * HSH:(c + 1) * HSH]),
        })
    res = bass_utils.run_bass_kernel_spmd(
        nc, in_maps, core_ids=list(range(NCORES)), trace=trace)
    out = np.empty((B, D_OUT), dtype=np.float32)
    for r in range(NCORES):
        o = res.results[r]["out_sh"]
        for c in range(RSCH):
            out[c * RSROWS + r * RSOUT:c * RSROWS + (r + 1) * RSOUT, :] = \
                o[c * RSOUT:(c + 1) * RSOUT, :]
    return out, res


def kernel(x, w1, w2, input_scale, hidden_scale):
    out, _ = run(x, w1, w2, input_scale, hidden_scale, trace=False)
    return out
